# revision 17
# baseline (speedup 1.0000x reference)
"""Trainium2 Bass kernel for nn_MessageFunction (GNN message passing).

Computes msg[b,o,n] = sum_d We[o,d]*e_vw[b,d,n] + sum_d Ww[o,d]*h_w[b,d,n]
                      + (be+bw)[o]
for B=128, D=768, N=256, data-parallel over B across 8 NeuronCores
(16 batches per core).

Design notes (all hardware-measured on trn2):
- fp16 matmuls with fp32 PSUM accumulation: full PE rate (f32r runs at
  1.25 cyc/col, fp16 at 1.0), rel err ~3e-4 at K=1536. Host casts the
  weights and activations to fp16; this also halves input HBM traffic.
- e and h are fused on host into one k-major slab [2*KT, 128, BPC*N]
  (the computation is [We Ww] @ [e; h]) so each block's activations
  arrive in a single 1.57MB DMA with 1KB contiguous runs.
- Outputs are written fp16 in m-major slabs [MT, 128, BPC*N] (1KB
  contiguous runs per partition) and reassembled + cast to f32 on host:
  halves store traffic vs f32.
- Loads ride the sync HWDGE ring, stores the scalar ring: HWDGE rings
  are FIFO per issuing engine, so stores (which depend on late compute)
  must never queue ahead of the next block's load.
- All weight loads are emitted before the timing loop; weights stay
  resident in SBUF (18KB/partition).
- 8 PSUM banks in flight (bufs=8) for the 576-matmul stream.
- For_i(staggered_reset=True): the default loop places an all-engine
  barrier in the per-iteration reset block, which drains the pipeline;
  staggered reset lets DMA prefetch run across the back-edge. The body
  is additionally unrolled 4x per For_i iteration (measured -7us/pass
  vs unroll=1 at sustained duty).
- Measured sustained floor for the bare 576-MM stream on this part is
  ~150-154us/pass (PE P0-throttles to ~1.9GHz under continuous load);
  the full kernel measures AT that floor (delta ~0.2us, within noise).

Experiment ledger (HW-measured at lr=1024 min-wall, do not retry blind):
  ADOPTED: weight loads hoisted out of For_i (baseline re-loaded 1.9MB
    every iteration w/ WAR stall); fused e+h input slab; fp16 m-slab
    outputs; loads=sync ring / stores=scalar ring; staggered_reset=True
    (-7us vs all-engine reset barrier); unroll=4 (-7us vs unroll=1,
    == unroll=8).
  REJECTED (delta vs base): korder=4 weight-reuse +18us; dve_split
    drain +26us; obufs=12 +3.5us; batch_store +3us; xbufs=4 +4us;
    stag0 +7us; hint_all +3us; bf16 inputs +1.7us; explicit ldweights
    0; 24-MM PSUM groups 0. fp8 fails precision (~5% >> 2e-2 gate).
  Timing: axon dispatch ~70-80ms/call drifts by ms; min-based (32,96)
    slopes can print garbage (observed 101us-797us for one binary).
    Use median-of-paired-diffs (test.py) or lr>=1024 min-wall deltas
    within one process. No NTFF hook here (antenv.axon_hooks missing);
    CoreSim no_exec perfetto (sim_trace.py) is the trace substitute.
"""
import numpy as np
import concourse.tile as tile
from concourse import bacc, mybir
from concourse.bass_utils import run_bass_kernel_spmd

try:  # persistent XLA cache: repeated fresh-process runs skip the NEFF compile
    import jax
    jax.config.update("jax_compilation_cache_dir", "/tmp/.jax_kernel_cache")
    jax.config.update("jax_persistent_cache_min_compile_time_secs", 0.5)
except Exception:
    pass

B, D, NN = 128, 768, 256
NCORES = 8
BPC = B // NCORES          # 16 batches per core
PAIR = 2                   # batches per 512-wide moving block
NBLK = BPC // PAIR         # 8 column blocks per pass
NCOL = PAIR * NN           # 512 moving columns
KT = 2 * D // 128          # 12 contraction tiles ([e; h] fused)
MT = D // 128              # 6 output row tiles
F32 = mybir.dt.float32
DT = mybir.dt.float16
NPDT = np.float16


KJ = KT // 2               # 6 double-k tiles for fp8 DoubleRow
F8 = mybir.dt.float8e4
SW = 256.0                 # weight quant scale (power of 2)
SX = 16.0                  # activation quant scale (power of 2)


def build(repeat: int = 1, loop_repeat: int = 1, stagger: bool = True,
          xbufs: int = 3, batch_store: bool = False, unroll: int = 8,
          hint_all: bool = False, obufs: int = 6, korder: int = 0,
          explicit_ldw: bool = False, dve_split: bool = False,
          bf16: bool = False, mode: str | None = None, nb: int = 1, **skw):
    mode = mode or MODE
    if mode.startswith("fp8"):
        return build_fp8(repeat=repeat, loop_repeat=loop_repeat,
                         stagger=stagger, unroll=unroll, mode=mode, nb=nb)
    if mode == "strassen":
        return build_strassen(repeat=repeat, loop_repeat=loop_repeat,
                              stagger=stagger, unroll=unroll, **skw)
    adt = mybir.dt.bfloat16 if bf16 else DT
    nc = bacc.Bacc("TRN2", target_bir_lowering=False, debug=False,
                   num_devices=NCORES)
    # activations arrive host-fused as [2*KT', 128, BPC*NN] fp16 k-slabs
    x = nc.dram_tensor("x", [KT, 128, BPC * NN], adt, kind="ExternalInput").ap()
    wT = nc.dram_tensor("wT", [2 * D, D], adt, kind="ExternalInput").ap()
    bias = nc.dram_tensor("bias", [D], F32, kind="ExternalInput").ap()
    out = nc.dram_tensor("out", [MT, 128, BPC * NN], DT,
                         kind="ExternalOutput").ap()

    wT_v = wT.rearrange("(k p) (m q) -> p k m q", p=128, q=128)  # [128,12,6,128]
    bias_v = bias.rearrange("(m p) -> p m", p=128)               # [128,6]

    with tile.TileContext(nc) as tc:
        with (
            tc.tile_pool(name="wpool", bufs=1) as wpool,
            tc.tile_pool(name="xpool", bufs=xbufs) as xpool,
            tc.tile_pool(name="opool", bufs=obufs) as opool,
            tc.tile_pool(name="pspool", bufs=8, space="PSUM") as pspool,
        ):
            w_t = wpool.tile([128, KT, MT, 128], adt)
            bias_t = wpool.tile([128, MT], F32)
            nc.sync.dma_start(bias_t[:], bias_v)
            nc.sync.dma_start(w_t[:], wT_v)

            def _block(c):
                xt = xpool.tile([128, KT, NCOL], adt, tag="xt", name="xt")
                cs = slice(c * NCOL, (c + 1) * NCOL)
                nc.sync.dma_start(xt[:], x[:, :, cs].rearrange("k p n -> p k n"))
                ot = (opool.tile([128, MT, NCOL], DT, name="ot")
                      if batch_store else None)
                for m in range(MT):
                    ps = pspool.tile([128, NCOL], F32, name="ps")
                    for k in range(KT):
                        nc.tensor.matmul(ps[:], w_t[:, k, m, :], xt[:, k, :],
                                         start=(k == 0), stop=(k == KT - 1))
                    res = ot[:, m, :] if batch_store else opool.tile(
                        [128, NCOL], DT, name="res")[:]
                    if dve_split and (m % 2 == 1):
                        # odd m: drain on the (otherwise idle) DVE so the
                        # ACT engine isn't the sole PSUM-drain path; store
                        # rides the sync ring to avoid stalling ACT's
                        # HWDGE queue on a cross-engine wait.
                        nc.vector.tensor_scalar_add(
                            res, ps[:], bias_t[:, m:m + 1])
                        if not batch_store:
                            nc.sync.dma_start(out[m, :, cs], res)
                    else:
                        nc.scalar.activation(
                            res, ps[:], mybir.ActivationFunctionType.Identity,
                            bias=bias_t[:, m:m + 1], scale=1.0)
                        if not batch_store:
                            nc.scalar.dma_start(out[m, :, cs], res)
                if batch_store:
                    nc.scalar.dma_start(
                        out[:, :, cs].rearrange("m p n -> p m n"), ot[:])

            def _khalf(h, nb):
                # k-outer order: nb blocks share each weight tile, so the
                # PE sees nb consecutive matmuls per LDWEIGHTS content.
                xt = xpool.tile([128, KT, nb * NCOL], adt, tag="xt", name="xt")
                cs = slice(h * nb * NCOL, (h + 1) * nb * NCOL)
                nc.sync.dma_start(xt[:], x[:, :, cs].rearrange("k p n -> p k n"))
                for m in range(MT):
                    pss = [pspool.tile([128, NCOL], F32, name="ps")
                           for _ in range(nb)]
                    for k in range(KT):
                        if explicit_ldw:
                            nc.tensor.ldweights(w_t[:, k, m, :])
                        for c in range(nb):
                            nc.tensor.matmul(
                                pss[c][:], w_t[:, k, m, :],
                                xt[:, k, c * NCOL:(c + 1) * NCOL],
                                start=(k == 0), stop=(k == KT - 1))
                    for c in range(nb):
                        res = opool.tile([128, NCOL], DT, name="res")
                        nc.scalar.activation(
                            res[:], pss[c][:],
                            mybir.ActivationFunctionType.Identity,
                            bias=bias_t[:, m:m + 1], scale=1.0)
                        nc.scalar.dma_start(
                            out[m, :, (h * nb + c) * NCOL:
                                (h * nb + c + 1) * NCOL], res[:])

            def body():
                for _ in range(repeat):
                    if korder:
                        for h in range(NBLK // korder):
                            _khalf(h, korder)
                    else:
                        for c in range(NBLK):
                            _block(c)

            hints = (tuple(mybir.ALL_ENGINES) if hint_all
                     else (mybir.EngineType.PE,))
            if loop_repeat > 1:
                if loop_repeat % unroll:
                    unroll = 1
                with tc.For_i(0, loop_repeat // unroll, 1,
                              staggered_reset=stagger, hint_engines=hints):
                    for _ in range(unroll):
                        body()
            else:
                body()
    nc.compile()
    return nc


def build_fp8(repeat: int = 1, loop_repeat: int = 1, stagger: bool = True,
              unroll: int = 4, mode: str = "fp8x3", xbufs: int = 3,
              obufs: int = 6, nb: int = 1):
    """3-term fp8 residual-split GEMM using DoubleRow perf mode.

    W = (Whi + Wlo)/SW, x = (xhi + xlo)/SX (all four factors e4m3);
    msg ~= (Whi@xhi + Wlo@xhi + Whi@xlo) / (SW*SX) + bias.
    Each DoubleRow matmul contracts two 128-row k-tiles at once.
    """
    nterm = {"fp8x1": 1, "fp8x2": 2, "fp8x3": 3}[mode]
    nc = bacc.Bacc("TRN2", target_bir_lowering=False, debug=False,
                   num_devices=NCORES)
    # x8[k, p, c, t, n]: k-tile k, partition p, block c, t=hi/lo, col n
    x8 = nc.dram_tensor("x8", [KT, 128, NBLK, 2, NCOL], F8,
                        kind="ExternalInput").ap()
    # wT8[h, kp, mq]: h=hi/lo of W_cat.T
    wT8 = nc.dram_tensor("wT8", [2, 2 * D, D], F8, kind="ExternalInput").ap()
    bias = nc.dram_tensor("bias", [D], F32, kind="ExternalInput").ap()
    out = nc.dram_tensor("out", [MT, 128, BPC * NN], DT,
                         kind="ExternalOutput").ap()

    # [p, h, j, t, m, q]: j = double-k tile, t = k-tile within pair
    wT_v = wT8.rearrange("h (j t p) (m q) -> p h j t m q", p=128, t=2, q=128)
    bias_v = bias.rearrange("(m p) -> p m", p=128)
    DR = mybir.MatmulPerfMode.DoubleRow

    with tile.TileContext(nc) as tc:
        with (
            tc.tile_pool(name="wpool", bufs=1) as wpool,
            tc.tile_pool(name="xpool", bufs=xbufs) as xpool,
            tc.tile_pool(name="opool", bufs=obufs) as opool,
            tc.tile_pool(name="pspool", bufs=8, space="PSUM") as pspool,
        ):
            w_t = wpool.tile([128, 2, KJ, 2, MT, 128], F8)
            bias_t = wpool.tile([128, MT], F32)
            nc.sync.dma_start(bias_t[:], bias_v)
            nc.sync.dma_start(w_t[:], wT_v)

            # (w_h, x_t) selectors per term: hi*hi, lo*hi, hi*lo
            terms = [(0, 0), (1, 0), (0, 1)][:nterm]

            def _block(c):
                xt = xpool.tile([128, KT, 2, NCOL], F8, tag="xt", name="xt")
                cs = slice(c * NCOL, (c + 1) * NCOL)
                nc.sync.dma_start(xt[:], x8[:, :, c].rearrange(
                    "k p t n -> p k t n"))
                n_mm = nterm * KJ
                for m in range(MT):
                    ps = pspool.tile([128, NCOL], F32, name="ps")
                    i = 0
                    for (wh, xsel) in terms:
                        for j in range(KJ):
                            nc.tensor.matmul(
                                ps[:], w_t[:, wh, j, :, m, :],
                                xt[:, 2 * j:2 * j + 2, xsel, :],
                                start=(i == 0), stop=(i == n_mm - 1),
                                perf_mode=DR)
                            i += 1
                    res = opool.tile([128, NCOL], DT, name="res")
                    nc.scalar.activation(
                        res[:], ps[:], mybir.ActivationFunctionType.Identity,
                        bias=bias_t[:, m:m + 1], scale=1.0 / (SW * SX))
                    nc.scalar.dma_start(out[m, :, cs], res[:])

            def _group(h):
                # nb consecutive blocks share each loaded weight pair: the
                # PE sees nb back-to-back matmuls per LDWEIGHTS content.
                xt = xpool.tile([128, KT, nb, 2, NCOL], F8, tag="xt",
                                name="xt")
                nc.sync.dma_start(xt[:], x8[:, :, h * nb:(h + 1) * nb]
                                  .rearrange("k p c t n -> p k c t n"))
                n_mm = nterm * KJ * nb
                for m in range(MT):
                    pss = [pspool.tile([128, NCOL], F32, name="ps")
                           for _ in range(nb)]
                    i = 0
                    for (wh, xsel) in terms:
                        for j in range(KJ):
                            for c in range(nb):
                                nc.tensor.matmul(
                                    pss[c][:], w_t[:, wh, j, :, m, :],
                                    xt[:, 2 * j:2 * j + 2, c, xsel, :],
                                    start=(i < nb), stop=(i >= n_mm - nb),
                                    perf_mode=DR)
                                i += 1
                    for c in range(nb):
                        res = opool.tile([128, NCOL], DT, name="res")
                        nc.scalar.activation(
                            res[:], pss[c][:],
                            mybir.ActivationFunctionType.Identity,
                            bias=bias_t[:, m:m + 1], scale=1.0 / (SW * SX))
                        nc.scalar.dma_start(
                            out[m, :, (h * nb + c) * NCOL:
                                (h * nb + c + 1) * NCOL], res[:])

            def body():
                for _ in range(repeat):
                    if nb > 1:
                        for h in range(NBLK // nb):
                            _group(h)
                    else:
                        for c in range(NBLK):
                            _block(c)

            if loop_repeat > 1:
                if loop_repeat % unroll:
                    unroll = 1
                with tc.For_i(0, loop_repeat // unroll, 1,
                              staggered_reset=stagger,
                              hint_engines=(mybir.EngineType.PE,)):
                    for _ in range(unroll):
                        body()
            else:
                body()
    nc.compile()
    return nc


KTH = KT // 2              # 6 k-tiles per Strassen K-half
NPAIR = NBLK // 2          # 4 column pairs (block c with block c+4)
# emission order M2, M5, M1, M3, M4, M6, M7 (raw-B products first so the
# PE can start each pair before the DVE B-combos land)
_PRODS = [1, 4, 0, 2, 3, 5, 6]


def build_strassen(repeat: int = 1, loop_repeat: int = 1,
                   stagger: bool = True, unroll: int = 8, xbufs: int = 2,
                   bbufs: int = 2, cbufs: int = 2, pipelined: bool = True,
                   nocombo: bool = False):
    """One level of Strassen on the per-core GEMM [768,1536]@[1536,4096].

    A-combos (weights) are free on the host; B-combos ride the DVE;
    each product drains PSUM->SBUF fp16 on ACT with a per-product bias
    beta_i chosen so the C-combos come out with the right (be+bw) bias
    with zero extra ops. C-combos + stores ride the DVE.
    PE work: 7/8 of the direct GEMM (504 matmuls/pass vs 576).
    """
    nc = bacc.Bacc("TRN2", target_bir_lowering=False, debug=False,
                   num_devices=NCORES)
    x = nc.dram_tensor("x", [KT, 128, BPC * NN], DT,
                       kind="ExternalInput").ap()
    a7 = nc.dram_tensor("a7", [7, D, D // 2], DT, kind="ExternalInput").ap()
    beta = nc.dram_tensor("beta", [7, D // 2], F32,
                          kind="ExternalInput").ap()
    out = nc.dram_tensor("out", [MT, 128, BPC * NN], DT,
                         kind="ExternalOutput").ap()

    a_v = a7.rearrange("i (k p) (m q) -> p i k m q", p=128, q=128)
    beta_v = beta.rearrange("i (m p) -> p i m", p=128)
    TTADD = mybir.AluOpType.add
    TTSUB = mybir.AluOpType.subtract

    with tile.TileContext(nc) as tc:
        with (
            tc.tile_pool(name="wpool", bufs=1) as wpool,
            tc.tile_pool(name="xpool", bufs=xbufs) as xpool,
            tc.tile_pool(name="bpool", bufs=bbufs) as bpool,
            tc.tile_pool(name="mpool", bufs=1) as mpool,
            tc.tile_pool(name="tpool", bufs=1) as tpool,
            tc.tile_pool(name="cpool", bufs=cbufs) as cpool,
            tc.tile_pool(name="pspool", bufs=8, space="PSUM") as pspool,
        ):
            w_t = wpool.tile([128, 7, KTH, 3, 128], DT)
            beta_t = wpool.tile([128, 7, 3], F32)
            nc.sync.dma_start(beta_t[:], beta_v)
            nc.sync.dma_start(w_t[:], a_v)

            def _prefetch(h):
                c1 = slice(h * NCOL, (h + 1) * NCOL)
                c2 = slice((h + NPAIR) * NCOL, (h + NPAIR + 1) * NCOL)
                xt = xpool.tile([128, 2, KT, NCOL], DT, name="xt")
                nc.sync.dma_start(
                    xt[:, 0], x[:, :, c1].rearrange("k p n -> p k n"))
                nc.sync.dma_start(
                    xt[:, 1], x[:, :, c2].rearrange("k p n -> p k n"))
                B11 = xt[:, 0, 0:KTH, :]
                B21 = xt[:, 0, KTH:, :]
                B12 = xt[:, 1, 0:KTH, :]
                B22 = xt[:, 1, KTH:, :]
                bc = bpool.tile([128, 5, KTH, NCOL], DT, name="bc")
                if not nocombo:
                    nc.vector.tensor_tensor(bc[:, 0], B11, B22, TTADD)
                    nc.vector.tensor_tensor(bc[:, 1], B12, B22, TTSUB)
                    nc.vector.tensor_tensor(bc[:, 2], B21, B11, TTSUB)
                    nc.vector.tensor_tensor(bc[:, 3], B11, B12, TTADD)
                    nc.vector.tensor_tensor(bc[:, 4], B21, B22, TTADD)
                return xt, bc

            def _compute(h, xt, bc):
                c1 = slice(h * NCOL, (h + 1) * NCOL)
                c2 = slice((h + NPAIR) * NCOL, (h + NPAIR + 1) * NCOL)
                B11 = xt[:, 0, 0:KTH, :]
                B21 = xt[:, 0, KTH:, :]
                B12 = xt[:, 1, 0:KTH, :]
                B22 = xt[:, 1, KTH:, :]
                if nocombo:   # timing diagnostic: same stream, no deps
                    movs = [B11, B11, B12, B21, B22, B11, B21]
                else:
                    movs = [bc[:, 0], B11, bc[:, 1], bc[:, 2], B22,
                            bc[:, 3], bc[:, 4]]
                mts = {}
                for i in _PRODS:
                    for m in range(3):
                        ps = pspool.tile([128, NCOL], F32, name="ps")
                        for k in range(KTH):
                            nc.tensor.matmul(
                                ps[:], w_t[:, i, k, m, :], movs[i][:, k, :],
                                start=(k == 0), stop=(k == KTH - 1))
                        mt = mpool.tile([128, NCOL], DT, name=f"mt{i}_{m}")
                        nc.scalar.activation(
                            mt[:], ps[:],
                            mybir.ActivationFunctionType.Identity,
                            bias=beta_t[:, i, m:m + 1], scale=1.0)
                        mts[(i, m)] = mt

                TT = nc.vector.tensor_tensor
                for m in range(3):   # C12 = M3 + M5 (ready after M3 drains)
                    c12 = cpool.tile([128, NCOL], DT, name=f"c12_{m}")
                    TT(c12[:], mts[(2, m)][:], mts[(4, m)][:], TTADD)
                    nc.scalar.dma_start(out[m, :, c2], c12[:])
                for m in range(3):   # chains ready after M4 drains
                    t1 = tpool.tile([128, NCOL], DT, name=f"t1_{m}")
                    TT(t1[:], mts[(0, m)][:], mts[(3, m)][:], TTADD)
                    t2 = tpool.tile([128, NCOL], DT, name=f"t2_{m}")
                    TT(t2[:], mts[(0, m)][:], mts[(1, m)][:], TTSUB)
                    c21 = cpool.tile([128, NCOL], DT, name=f"c21_{m}")
                    TT(c21[:], mts[(1, m)][:], mts[(3, m)][:], TTADD)
                    nc.scalar.dma_start(out[m + 3, :, c1], c21[:])
                    t4 = tpool.tile([128, NCOL], DT, name=f"t4_{m}")
                    TT(t4[:], t2[:], mts[(2, m)][:], TTADD)
                    t5 = tpool.tile([128, NCOL], DT, name=f"t5_{m}")
                    TT(t5[:], t1[:], mts[(4, m)][:], TTSUB)
                    mts[(0, m)] = (t1, t2)   # keep refs alive
                    mts[(3, m)] = (t4, t5)
                for m in range(3):   # C22 = t4 + M6 (after M6 drains)
                    c22 = cpool.tile([128, NCOL], DT, name=f"c22_{m}")
                    TT(c22[:], mts[(3, m)][0][:], mts[(5, m)][:], TTADD)
                    nc.scalar.dma_start(out[m + 3, :, c2], c22[:])
                for m in range(3):   # C11 = t5 + M7 (after M7 drains)
                    c11 = cpool.tile([128, NCOL], DT, name=f"c11_{m}")
                    TT(c11[:], mts[(3, m)][1][:], mts[(6, m)][:], TTADD)
                    nc.scalar.dma_start(out[m, :, c1], c11[:])

            cur = [None]
            if pipelined:
                cur[0] = _prefetch(0)   # prologue (outside any For_i)

            def body():
                for _ in range(repeat):
                    if pipelined:
                        # next pair's loads + B-combos are emitted ahead of
                        # this pair's products, so the DVE queue never makes
                        # the PE wait at a pair boundary.
                        for h in range(NPAIR):
                            nxt = _prefetch((h + 1) % NPAIR)
                            _compute(h, *cur[0])
                            cur[0] = nxt
                    else:
                        for h in range(NPAIR):
                            _compute(h, *_prefetch(h))

            if loop_repeat > 1:
                if loop_repeat % unroll:
                    unroll = 1
                with tc.For_i(0, loop_repeat // unroll, 1,
                              staggered_reset=stagger,
                              hint_engines=(mybir.EngineType.PE,)):
                    for _ in range(unroll):
                        body()
            else:
                body()
    nc.compile()
    return nc


def _prep_in_maps_strassen(h_w, e_vw, We, be, Ww, bw):
    wT = np.concatenate([np.asarray(We, np.float32).T,
                         np.asarray(Ww, np.float32).T], axis=0)
    T11, T12 = wT[:D, :D // 2], wT[D:, :D // 2]     # A11^T, A12^T
    T21, T22 = wT[:D, D // 2:], wT[D:, D // 2:]     # A21^T, A22^T
    a7 = np.stack([T11 + T22, T21 + T22, T11, T22, T11 + T12,
                   T21 - T11, T12 - T22]).astype(NPDT)
    bias = np.asarray(be, np.float32) + np.asarray(bw, np.float32)
    b_u, b_l = bias[:D // 2], bias[D // 2:]
    z = np.zeros_like(b_u)
    beta = np.ascontiguousarray(np.stack(
        [b_u - b_l, z, b_u, b_l, z, 2.0 * (b_l - b_u), z]))

    kt_half = KT // 2
    e_vw = np.asarray(e_vw, np.float32).astype(NPDT)
    h_w = np.asarray(h_w, np.float32).astype(NPDT)

    def slab(xx, c):
        s = xx[c * BPC:(c + 1) * BPC].reshape(BPC, kt_half, 128, NN)
        return s.transpose(1, 2, 0, 3).reshape(kt_half, 128, BPC * NN)

    return [
        {"x": np.ascontiguousarray(
            np.concatenate([slab(e_vw, c), slab(h_w, c)], axis=0)),
         "a7": a7, "beta": beta}
        for c in range(NCORES)
    ]
    import ml_dtypes
    f8 = np.dtype(ml_dtypes.float8_e4m3)
    wcatT = np.concatenate([np.asarray(We, np.float32).T,
                            np.asarray(Ww, np.float32).T], axis=0) * SW
    whi = wcatT.astype(f8)
    wlo = (wcatT - whi.astype(np.float32)).astype(f8)
    wT8 = np.ascontiguousarray(np.stack([whi, wlo], axis=0))
    bias = (np.asarray(be, np.float32) + np.asarray(bw, np.float32))

    kt_half = KT // 2
    e_vw = np.asarray(e_vw, np.float32)
    h_w = np.asarray(h_w, np.float32)

    def slab(xx, c):
        # [BPC, D, NN] -> [KT/2, 128, BPC*NN]
        s = xx[c * BPC:(c + 1) * BPC].reshape(BPC, kt_half, 128, NN)
        return s.transpose(1, 2, 0, 3).reshape(kt_half, 128, BPC * NN)

    maps = []
    for c in range(NCORES):
        s = np.concatenate([slab(e_vw, c), slab(h_w, c)], axis=0) * SX
        shi = s.astype(f8)
        slo = (s - shi.astype(np.float32)).astype(f8)
        # [KT, 128, BPC*NN] -> [KT, 128, NBLK, 2, NCOL]
        x8 = np.empty((KT, 128, NBLK, 2, NCOL), dtype=f8)
        x8[:, :, :, 0, :] = shi.reshape(KT, 128, NBLK, NCOL)
        x8[:, :, :, 1, :] = slo.reshape(KT, 128, NBLK, NCOL)
        maps.append({"x8": np.ascontiguousarray(x8), "wT8": wT8,
                     "bias": bias})
    return maps


def _prep_in_maps(h_w, e_vw, We, be, Ww, bw, bf16=False):
    if bf16:
        import ml_dtypes
        npdt = np.dtype(ml_dtypes.bfloat16)
    else:
        npdt = NPDT
    e_vw = np.asarray(e_vw, dtype=np.float32).astype(npdt)
    h_w = np.asarray(h_w, dtype=np.float32).astype(npdt)
    # [We Ww] @ [e; h]: stationary operand is W_cat.T = vstack(We.T, Ww.T)
    wT = np.ascontiguousarray(
        np.concatenate([np.asarray(We, dtype=np.float32).T,
                        np.asarray(Ww, dtype=np.float32).T],
                       axis=0)).astype(npdt)
    bias = (np.asarray(be, dtype=np.float32)
            + np.asarray(bw, dtype=np.float32)).astype(np.float32)

    kt_half = KT // 2

    def slab(xx, c):
        # [BPC, D, NN] -> [KT/2, 128, BPC*NN] : s[k, p, b*NN+n] = xx[b, k*128+p, n]
        s = xx[c * BPC:(c + 1) * BPC].reshape(BPC, kt_half, 128, NN)
        return s.transpose(1, 2, 0, 3).reshape(kt_half, 128, BPC * NN)

    return [
        {"x": np.ascontiguousarray(
            np.concatenate([slab(e_vw, c), slab(h_w, c)], axis=0)),
         "wT": wT, "bias": bias}
        for c in range(NCORES)
    ]


def _unpack_out(o):
    # [MT, 128, NBLK*PAIR*NN] fp16 -> [BPC, D, NN] f32
    # o[m, p, c*NCOL + pb*NN + n] = msg[c*PAIR+pb, m*128+p, n]
    return np.ascontiguousarray(
        o.reshape(MT, 128, NBLK, PAIR, NN)
         .transpose(2, 3, 0, 1, 4)
         .reshape(BPC, D, NN)).astype(np.float32)


_NC_CACHE = []

MODE = "strassen"


def prep_in_maps(h_w, e_vw, We, be, Ww, bw, mode=None):
    mode = mode or MODE
    if mode == "strassen":
        return _prep_in_maps_strassen(h_w, e_vw, We, be, Ww, bw)
    if mode.startswith("fp8"):
        return _prep_in_maps_fp8(h_w, e_vw, We, be, Ww, bw)
    return _prep_in_maps(h_w, e_vw, We, be, Ww, bw)


def kernel(h_v, h_w, e_vw, We, be, Ww, bw):
    if not _NC_CACHE:
        _NC_CACHE.append(build(mode=MODE))
    nc = _NC_CACHE[0]
    in_maps = prep_in_maps(h_w, e_vw, We, be, Ww, bw)
    r = run_bass_kernel_spmd(nc, in_maps, core_ids=list(range(NCORES)))
    return np.concatenate(
        [_unpack_out(r.results[c]["out"]) for c in range(NCORES)], axis=0)

